# revision 1
# baseline (speedup 1.0000x reference)
"""Trainium2 Bass kernel for nn_Cam_59785944760667 (gated GCN, 3 layers).

Self-contained: takes FULL inputs, shards across 8 NeuronCores internally,
returns the FULL [N, C] output.

Design:
  - Nodes sharded contiguously across 8 cores (12500/core, padded to 12544).
  - Symmetric GCN normalization is separable: val = dn[col]*dn[row].
    dn[row] is folded into the gathered table (g = dn * h, recomputed per
    layer on-device); dn[col] is folded into the per-edge one-hot rows
    (host-precomputed constants).
  - Per layer: AllGather g across cores -> per-core DRAM table; per-edge
    source rows gathered with gpsimd.dma_gather (int16 indices, so the
    table is addressed in 4 quarters of 25088 rows); segment-sum into
    feat-major agg^T via one-hot matmuls accumulating in PSUM per
    128-dest-node block; dense gating + K-head einsum on TensorE.
  - Edge layout: per (block, quarter) runs padded to 128-edge tiles with a
    core-uniform template (SPMD program is shared across cores).
"""
import time
from contextlib import ExitStack

import numpy as np

# problem constants
N, D, H, K, L, C = 100000, 128, 64, 8, 3, 16
E = 1600000
THETA = 0.1

# sharding constants
NCORES = 8
SH = N // NCORES          # 12500 real nodes per core
BLK = 128
NB = (SH + BLK - 1) // BLK  # 98 blocks
SHP = NB * BLK            # 12544 padded shard rows
NQ = 4
QROWS = 2 * SHP           # 25088 table rows per quarter (fits int16)
CHT = 24                  # gather-chunk size in 128-edge tiles

_CACHE = {}


# ---------------------------------------------------------------- host prep
def _prep(edge_index, dn):
    """Build the core-uniform edge template and per-core gather/one-hot data.

    Returns dict with:
      T        [NB, NQ]  tiles per (block, quarter)  (same for all cores)
      ntiles_q [NQ]      tiles per quarter stream
      nch_q    [NQ]      gather chunks per quarter stream
      idx      [NC][128, sum_q nch_q*CHT*8] int16 wrapped gather indices
      colc     [NC][128, NTP] fp32 one-hot column (or -1 for pads)
      dnec     [NC][128, NTP] fp32 dn[col] per edge (0 for pads)
    where NTP = sum_q nch_q*CHT (padded tile count, uniform).
    """
    row = edge_index[0].astype(np.int64)
    col = edge_index[1].astype(np.int64)

    core_of = col // SH
    r = col % SH
    b_of = r // BLK
    p_of = r % BLK
    srcg = (row // SH) * SHP + (row % SH)
    q_of = srcg // QROWS
    lidx = srcg % QROWS

    key = (core_of * NB + b_of) * NQ + q_of
    cnt = np.bincount(key, minlength=NCORES * NB * NQ).reshape(NCORES, NB, NQ)
    T = np.maximum(1, np.ceil(cnt.max(axis=0) / BLK)).astype(np.int64)  # [NB, NQ]

    ntiles_q = T.sum(axis=0)                      # [NQ]
    off = np.zeros((NB, NQ), np.int64)            # tile offset of (b, q) in stream q
    off[1:] = np.cumsum(T, axis=0)[:-1]
    nch_q = np.ceil(ntiles_q / CHT).astype(np.int64)
    ntp_q = nch_q * CHT                           # padded tiles per stream
    NTP = int(ntp_q.sum())

    idx_all, colc_all, dnec_all = [], [], []
    for c in range(NCORES):
        m = core_of == c
        qc, bc, lc, pc = q_of[m], b_of[m], lidx[m], p_of[m]
        dnc = dn[col[m]]
        order = np.lexsort((bc, qc))
        qs, bs, ls, ps, ds = (a[order] for a in (qc, bc, lc, pc, dnc))
        gk = qs * NB + bs
        first = np.searchsorted(gk, gk)           # first index of own group
        rank = np.arange(gk.size) - first
        slot = off[bs, qs] * BLK + rank           # position within stream q

        streams_i, streams_c, streams_d = [], [], []
        for q in range(NQ):
            npad = int(ntp_q[q]) * BLK
            si = np.zeros(npad, np.int16)
            sc = np.full(npad, -1.0, np.float32)
            sd = np.zeros(npad, np.float32)
            mq = qs == q
            si[slot[mq]] = ls[mq].astype(np.int16)
            sc[slot[mq]] = ps[mq].astype(np.float32)
            sd[slot[mq]] = ds[mq].astype(np.float32)
            streams_i.append(si)
            streams_c.append(sc)
            streams_d.append(sd)

        # wrapped idx: per chunk [16, CHT*8] -> replicate to [128, CHT*8]
        wcols = []
        for q in range(NQ):
            si = streams_i[q].reshape(int(nch_q[q]), CHT * BLK)
            for k in range(int(nch_q[q])):
                w = si[k].reshape(CHT * BLK // 16, 16).T      # [16, CHT*8]
                wcols.append(np.tile(w, (8, 1)))
        idx_all.append(np.concatenate(wcols, axis=1))          # [128, sum nch*CHT*8]

        colc = np.concatenate(
            [streams_c[q].reshape(int(ntp_q[q]), BLK).T for q in range(NQ)], axis=1)
        dnec = np.concatenate(
            [streams_d[q].reshape(int(ntp_q[q]), BLK).T for q in range(NQ)], axis=1)
        colc_all.append(np.ascontiguousarray(colc, np.float32))   # [128, NTP]
        dnec_all.append(np.ascontiguousarray(dnec, np.float32))

    return dict(T=T, off=off, ntiles_q=ntiles_q, nch_q=nch_q, ntp_q=ntp_q,
                NTP=NTP, idx=idx_all, colc=colc_all, dnec=dnec_all)


# ---------------------------------------------------------------- device prog
def _build(tpl, dt_g):
    import concourse.bass as bass
    import concourse.tile as tile
    from concourse import bacc, mybir
    from concourse._compat import with_exitstack
    from concourse.bass import _add_dep_helper
    from concourse.masks import make_identity
    from concourse.library_config import mlp

    f32 = mybir.dt.float32
    i16 = mybir.dt.int16
    Alu = mybir.AluOpType
    Act = mybir.ActivationFunctionType

    T, off = tpl["T"], tpl["off"]
    nch_q, ntp_q, NTP = tpl["nch_q"], tpl["ntp_q"], tpl["NTP"]
    GTROWS = NCORES * SHP     # 100352

    nc = bacc.Bacc("TRN2", target_bir_lowering=False, debug=False,
                   num_devices=NCORES)
    P = {}  # dram params

    def par(name, shape, dtype=f32, out=False):
        P[name] = nc.declare_dram_parameter(name, list(shape), dtype,
                                            isOutput=out).ap()
        return P[name]

    xT = par("xT", [128, SHP])
    idx = par("idx", [128, int(nch_q.sum()) * CHT * 8], i16)
    colc = par("colc", [128, NTP])
    dnec = par("dnec", [128, NTP])
    dn_n = par("dn_n", [128, NB])
    iota = par("iota", [128, 128])
    fc0w = par("fc0w", [D, H])
    fc0b = par("fc0b", [128, H])
    fc1w = par("fc1w", [H, C])
    fc1b = par("fc1b", [128, C])
    envw = par("envw", [L * H, K])
    envb = par("envb", [L * 128, K])
    wstk = par("wstk", [L * 128, K * H])
    out_p = par("out", [SHP, C], out=True)

    # internal DRAM: per-layer g shard + gathered table
    g_shard = [nc.dram_tensor(f"g_shard{l}", [SHP, H], dt_g) for l in range(L)]
    g_table = [nc.dram_tensor(f"g_table{l}", [GTROWS, H], dt_g,
                              addr_space="Shared") for l in range(L)]

    @with_exitstack
    def prog(ctx: ExitStack, tc: tile.TileContext):
        sb = ctx.enter_context(tc.tile_pool(name="persist", bufs=1))
        chunks = ctx.enter_context(tc.tile_pool(name="chunks", bufs=8))
        work = ctx.enter_context(tc.tile_pool(name="work", bufs=3))
        oh_p = ctx.enter_context(tc.tile_pool(name="oh", bufs=3))
        psA = ctx.enter_context(tc.tile_pool(name="psA", bufs=2, space="PSUM"))
        psB = ctx.enter_context(tc.tile_pool(name="psB", bufs=2, space="PSUM"))
        psC = ctx.enter_context(tc.tile_pool(name="psC", bufs=2, space="PSUM"))

        # ---- persistent SBUF loads
        def load(name, shape, dtype=f32, src=None):
            t = sb.tile(shape, dtype, tag=name)
            nc.sync.dma_start(out=t[:], in_=(src if src is not None else P[name])[:])
            return t

        idx_sb = load("idx", [128, int(nch_q.sum()) * CHT * 8], i16)
        colc_sb = load("colc", [128, NTP])
        dnec_sb = load("dnec", [128, NTP])
        dn_sb = load("dn_n", [128, NB])
        iota_sb = load("iota", [128, 128])
        fc0w_sb = load("fc0w", [D, H])
        fc0b_sb = load("fc0b", [128, H])
        fc1w_sb = load("fc1w", [H, C])
        fc1b_sb = load("fc1b", [128, C])
        envw_sb = load("envw", [L * H, K])
        envb_sb = load("envb", [L * 128, K])
        wstk_sb = load("wstk", [L * 128, K * H])
        ident = sb.tile([128, 128], f32, tag="ident")
        make_identity(nc, ident[:])

        h_a = sb.tile([128, NB * H], f32, tag="h_a")
        h_b = sb.tile([128, NB * H], f32, tag="h_b")

        lib = nc.gpsimd.load_library(mlp)

        # ---- fc0: h0 = relu(x @ fc0_w + b), g0 = dn*h0
        g_dma = {l: [] for l in range(L)}
        for b in range(NB):
            xt = work.tile([128, 128], f32, tag="xT")
            nc.sync.dma_start(out=xt[:], in_=xT[:, b * 128:(b + 1) * 128])
            ps = psB.tile([128, H], f32, tag="fc0ps", space="PSUM")
            nc.tensor.matmul(out=ps[:], lhsT=xt[:], rhs=fc0w_sb[:],
                             start=True, stop=True)
            hb = h_a[:, b * H:(b + 1) * H]
            nc.vector.tensor_tensor(out=hb, in0=ps[:], in1=fc0b_sb[:], op=Alu.add)
            nc.scalar.activation(hb, hb, Act.Relu)
            gt = work.tile([128, H], dt_g, tag="gtile")
            nc.vector.tensor_scalar(gt[:], hb, dn_sb[:, b:b + 1], None, Alu.mult)
            d = nc.sync.dma_start(out=g_shard[0][:].rearrange(
                "(b p) h -> p (b h)", p=128)[:, b * H:(b + 1) * H], in_=gt[:])
            g_dma[0].append(d)

        # stream-q chunk bookkeeping
        qtile_base = np.zeros(NQ, np.int64)      # tile offset of stream q in consts
        qtile_base[1:] = np.cumsum(ntp_q)[:-1]
        qidx_base = np.zeros(NQ, np.int64)       # idx col offset of stream q
        qidx_base[1:] = np.cumsum(nch_q * CHT * 8)[:-1]

        cur = [h_a, h_b]
        for l in range(L):
            cc = nc.gpsimd.collective_compute(
                "AllGather", Alu.bypass,
                replica_groups=[[i for i in range(NCORES)]],
                ins=[g_shard[l][:]],
                outs=[g_table[l][:]],
            )
            for d in g_dma[l]:
                _add_dep_helper(cc.ins, d.ins, True, "cc waits g writes")

            h_cur, h_nxt = cur[l % 2], cur[(l + 1) % 2]
            chunk_tiles = {}

            def get_chunk(q, k, l=l, cc=cc, chunk_tiles=chunk_tiles):
                if (q, k) in chunk_tiles:
                    return chunk_tiles[(q, k)]
                xt = chunks.tile([128, CHT * H], dt_g, tag="chunk")
                ic0 = int(qidx_base[q] + k * CHT * 8)
                g = nc.gpsimd.dma_gather(
                    out_ap=xt[:].rearrange("p (t h) -> p t h", h=H),
                    in_ap=g_table[l][int(q * QROWS):, :],
                    idxs_ap=idx_sb[:, ic0:ic0 + CHT * 8],
                    num_idxs=CHT * 128,
                    num_idxs_reg=CHT * 128,
                    elem_size=H,
                )
                _add_dep_helper(g.ins, lib.ins, True, "lib before gather")
                _add_dep_helper(g.ins, cc.ins, True, "gather waits allgather")
                chunk_tiles[(q, k)] = xt
                return xt

            for b in range(NB):
                hiT_ps = psA.tile([128, 128], f32, tag="hiT", space="PSUM")
                # h^T at partitions 0..63
                nc.tensor.transpose(out=hiT_ps[0:64, :],
                                    in_=h_cur[:, b * H:(b + 1) * H],
                                    identity=ident[:])
                # agg^T accumulation at partitions 64..127
                nmm = int(T[b].sum())
                mm_i = 0
                for q in range(NQ):
                    for t in range(int(T[b, q])):
                        s = int(off[b, q]) + t            # stream tile
                        k, sl = s // CHT, s % CHT
                        xt = get_chunk(q, k)
                        tg = int(qtile_base[q]) + s       # const column
                        oh = oh_p.tile([128, 128], dt_g, tag="oh")
                        nc.vector.tensor_scalar(
                            oh[:], iota_sb[:], colc_sb[:, tg:tg + 1],
                            dnec_sb[:, tg:tg + 1], Alu.is_equal, Alu.mult)
                        nc.tensor.matmul(
                            out=hiT_ps[64:128, :],
                            lhsT=xt[:, sl * H:(sl + 1) * H],
                            rhs=oh[:],
                            start=(mm_i == 0), stop=(mm_i == nmm - 1))
                        mm_i += 1
                hiT = work.tile([128, 128], f32, tag="hiT_sb")
                nc.vector.tensor_copy(hiT[:], hiT_ps[:])

                # gate
                gps = psC.tile([128, K], f32, tag="gate", space="PSUM")
                nc.tensor.matmul(out=gps[:], lhsT=hiT[0:64, :],
                                 rhs=envw_sb[l * H:(l + 1) * H, :],
                                 start=True, stop=True)
                gx = work.tile([128, K], f32, tag="gx")
                nc.vector.tensor_tensor(out=gx[:], in0=gps[:],
                                        in1=envb_sb[l * 128:(l + 1) * 128, :],
                                        op=Alu.add)
                gm = work.tile([128, 1], f32, tag="gm")
                nc.vector.tensor_reduce(out=gm[:], in_=gx[:],
                                        axis=mybir.AxisListType.X, op=Alu.max)
                nc.vector.tensor_scalar(gm[:], gm[:], -1.0, None, Alu.mult)
                ge = work.tile([128, K], f32, tag="ge")
                nc.scalar.activation(ge[:], gx[:], Act.Exp, bias=gm[:, 0:1])
                gs = work.tile([128, 1], f32, tag="gs")
                nc.vector.tensor_reduce(out=gs[:], in_=ge[:],
                                        axis=mybir.AxisListType.X, op=Alu.add)
                gr = work.tile([128, 1], f32, tag="gr")
                nc.vector.reciprocal(gr[:], gs[:])
                nc.vector.tensor_scalar(gs[:], gs[:], THETA, None, Alu.mult)
                gmask = work.tile([128, K], f32, tag="gmask")
                nc.vector.tensor_scalar(gmask[:], ge[:], gs[:, 0:1], None, Alu.is_gt)
                nc.vector.tensor_tensor(out=gmask[:], in0=gmask[:], in1=ge[:],
                                        op=Alu.mult)
                nc.vector.tensor_scalar(gmask[:], gmask[:], gr[:, 0:1], None,
                                        Alu.mult)

                # einsum
                tps = psB.tile([128, K * H], f32, tag="tmp", space="PSUM")
                nc.tensor.matmul(out=tps[:], lhsT=hiT[:],
                                 rhs=wstk_sb[l * 128:(l + 1) * 128, :],
                                 start=True, stop=True)
                msk = work.tile([128, K * H], f32, tag="msk")
                nc.vector.tensor_tensor(
                    out=msk[:].rearrange("p (k o) -> p k o", k=K),
                    in0=tps[:].rearrange("p (k o) -> p k o", k=K),
                    in1=gmask[:].to_broadcast([128, K, H]),
                    op=Alu.mult)
                ob = work.tile([128, H], f32, tag="ob")
                nc.vector.tensor_reduce(
                    out=ob[:], in_=msk[:].rearrange("p (k o) -> p o k", k=K),
                    axis=mybir.AxisListType.X, op=Alu.add)
                # residual + relu
                hn = h_nxt[:, b * H:(b + 1) * H]
                nc.vector.tensor_tensor(out=hn, in0=ob[:],
                                        in1=h_cur[:, b * H:(b + 1) * H], op=Alu.add)
                nc.scalar.activation(hn, hn, Act.Relu)

                if l < L - 1:
                    gt = work.tile([128, H], dt_g, tag="gtile")
                    nc.vector.tensor_scalar(gt[:], hn, dn_sb[:, b:b + 1], None,
                                            Alu.mult)
                    d = nc.sync.dma_start(out=g_shard[l + 1][:].rearrange(
                        "(b p) h -> p (b h)", p=128)[:, b * H:(b + 1) * H],
                        in_=gt[:])
                    g_dma[l + 1].append(d)
                else:
                    # fc1 fused
                    h2ps = psC.tile([64, 128], f32, tag="h2T", space="PSUM")
                    nc.tensor.transpose(out=h2ps[:], in_=hn, identity=ident[:])
                    h2 = work.tile([64, 128], f32, tag="h2sb")
                    nc.vector.tensor_copy(h2[:], h2ps[:])
                    ops_ = psB.tile([128, C], f32, tag="ops", space="PSUM")
                    nc.tensor.matmul(out=ops_[:], lhsT=h2[:], rhs=fc1w_sb[:],
                                     start=True, stop=True)
                    ot = work.tile([128, C], f32, tag="ot")
                    nc.vector.tensor_tensor(out=ot[:], in0=ops_[:], in1=fc1b_sb[:],
                                            op=Alu.add)
                    nc.sync.dma_start(out=out_p[:].rearrange(
                        "(b p) c -> p (b c)", p=128)[:, b * C:(b + 1) * C],
                        in_=ot[:])

    with tile.TileContext(nc, num_cores=NCORES) as tc:
        prog(tc)
    nc.compile()
    return nc


# ---------------------------------------------------------------- entry point
def kernel(**inputs):
    from concourse.bass_utils import run_bass_kernel_spmd

    x = np.ascontiguousarray(np.asarray(inputs["x"], np.float32))
    ei = np.asarray(inputs["edge_index"], np.int64)
    fc0_w = np.asarray(inputs["fc0_w"], np.float32)
    fc0_b = np.asarray(inputs["fc0_b"], np.float32)
    fc1_w = np.asarray(inputs["fc1_w"], np.float32)
    fc1_b = np.asarray(inputs["fc1_b"], np.float32)
    env_w = np.asarray(inputs["env_w"], np.float32)
    env_b = np.asarray(inputs["env_b"], np.float32)
    conv_w = np.asarray(inputs["conv_w"], np.float32)

    deg = np.bincount(ei[1], minlength=N).astype(np.float32)
    dn = np.where(deg > 0, 1.0 / np.sqrt(deg), 0.0).astype(np.float32)

    key = "prog"
    if key not in _CACHE:
        tpl = _prep(ei, dn)
        from concourse import mybir
        nc = _build(tpl, mybir.dt.float32)
        _CACHE[key] = (tpl, nc)
    tpl, nc = _CACHE[key]

    # weight transforms (host)
    permf = np.concatenate([np.arange(H, 2 * H), np.arange(0, H)])  # ours->ref row
    wstk = np.stack([
        conv_w[l][:, permf, :].transpose(1, 0, 2).reshape(2 * H, K * H)
        for l in range(L)]).reshape(L * 2 * H, K * H).astype(np.float32)
    envw = env_w[:, :H, :].reshape(L * H, K).astype(np.float32)
    envb = np.concatenate([np.tile(env_b[l][None, :], (128, 1))
                           for l in range(L)]).astype(np.float32)
    fc0b_rep = np.tile(fc0_b[None, :], (128, 1)).astype(np.float32)
    fc1b_rep = np.tile(fc1_b[None, :], (128, 1)).astype(np.float32)
    iota = np.tile(np.arange(128, dtype=np.float32)[None, :], (128, 1))

    in_maps = []
    for c in range(NCORES):
        sl = slice(c * SH, (c + 1) * SH)
        xs = np.zeros((SHP, D), np.float32)
        xs[:SH] = x[sl]
        dnn = np.zeros((128, NB), np.float32)
        dnv = np.zeros(SHP, np.float32)
        dnv[:SH] = dn[sl]
        dnn[:, :] = dnv.reshape(NB, 128).T
        in_maps.append(dict(
            xT=np.ascontiguousarray(xs.T),
            idx=tpl["idx"][c],
            colc=tpl["colc"][c],
            dnec=tpl["dnec"][c],
            dn_n=np.ascontiguousarray(dnn),
            iota=iota,
            fc0w=fc0_w, fc0b=fc0b_rep, fc1w=fc1_w, fc1b=fc1b_rep,
            envw=envw, envb=envb, wstk=wstk,
        ))

    t0 = time.time()
    res = run_bass_kernel_spmd(nc, in_maps, list(range(NCORES)))
    t1 = time.time()
    kernel.last_run_s = t1 - t0

    out = np.empty((N, C), np.float32)
    for c in range(NCORES):
        oc = res.results[c]["out"].reshape(SHP, C)
        out[c * SH:(c + 1) * SH] = oc[:SH]
    return out
